# revision 100
# baseline (speedup 1.0000x reference)
"""Mamba block (LN1 -> Mamba -> +res -> LN2 -> FFN -> +res) on 8 trn2 cores.

Sharding: core c handles batch b=c//4 and d_inner shard s=c%4 (512 channels)
for the Mamba part; token slice s*256:(s+1)*256 of its batch for the FFN.
Two collectives per 4-core batch group: AllReduce of x_proj partials
([96,1024] fp32) and ReduceScatter of out_proj partials (arranged [4,DM,TS]
so each core receives exactly its token slice). Everything stays
channel-major on device (time on the free axis — required by the
tensor_tensor_scan recurrence); the host pre-transposes x and all weights.
Matmuls run in bf16 with fp32 PSUM accumulation; LN/scan/elementwise in fp32.

Scan engine split: Act computes dA=exp(dt*A); DVE runs the scan recurrences
(TensorScalarPtr is DVE-only on HW) plus the dBx muls; most C*h muls go to
Pool as plain TensorTensor; the sum over the 16 states runs on the Tensor
engine as identity-matmul PSUM accumulation (PE is otherwise idle here).
The depthwise conv also runs on PE as diag(w_k) matmuls PSUM-accumulated
over the 4 taps.
"""
import numpy as np
import ml_dtypes

import concourse.bass as bass
import concourse.bacc as bacc
import concourse.tile as tile
from concourse import mybir

f32 = mybir.dt.float32
bf16 = mybir.dt.bfloat16
AF = mybir.ActivationFunctionType
OP = mybir.AluOpType
BF = ml_dtypes.bfloat16

# problem shapes (hardcoded per contract)
B, L, DM = 2, 1024, 1024
DI, NST, DCONV, DTR = 2048, 16, 4, 64
FFH = 4 * DM                       # 4096
NCORES, GROUP = 8, 4
DS = DI // GROUP                   # 512 channels per core
TS = L // GROUP                    # 256 tokens per core for FFN
P = 128
DMT = DM // P                      # 8
DST = DS // P                      # 4
FFT = FFH // P                     # 32
NH = L // 512                      # 2 halves of the free dim for matmul N<=512
NXD = DTR + 2 * NST                # 96
EPS = 1e-5
REPLICA_GROUPS = [[0, 1, 2, 3], [4, 5, 6, 7]]

# packed per-partition scalar params: name -> (offset, ncols)
_SM_OFF = {}
_off = 0
for _nm, _nc_ in [("cb", DST), ("dtb", DST), ("Dvec", DST), ("ln1g", DMT),
                  ("ln1b", DMT), ("ln2g", DMT), ("ln2b", DMT), ("b2", DMT),
                  ("cw", DST * DCONV), ("Amat", DST * NST), ("b1", FFT),
                  ("vneg", 2 * DST), ("w0z", DST), ("ident", P)]:
    _SM_OFF[_nm] = (_off, _nc_)
    _off += _nc_
SM_COLS = _off


def _bcast_row(t, row, n):
    """AP broadcasting DRAM row t[row, :n] across P partitions."""
    r = t[row:row + 1, 0:n]
    return bass.AP(tensor=r.tensor, offset=r.offset, ap=[[0, P], [1, n]])


def build_nc(num_devices=NCORES, replica_groups=REPLICA_GROUPS):
    nc = bacc.Bacc("TRN2", target_bir_lowering=False, debug=False,
                   enable_asserts=True, num_devices=num_devices)
    D = {}

    def inp(name, shape, dt=f32):
        D[name] = nc.dram_tensor(name, shape, dt, kind="ExternalInput")
        return D[name]

    inp("xT", [DM, L])                    # x[b].T
    inp("xT_bf", [DM, L], bf16)           # raw x bf16: in_proj rhs + stats
    inp("xTs", [DM, TS])                  # x[b].T[:, token slice] (residual)
    inp("w_in", [2 * DS, DM], bf16)       # in_proj lhsT, m-tiled rows
    inp("w_x", [DS, NXD], bf16)           # lhsT for x_proj
    inp("w_dt", [DTR, DS], bf16)          # lhsT for dt_proj
    inp("w_out", [DS, DM], bf16)          # lhsT for out_proj
    inp("smalls", [P, SM_COLS])           # packed per-partition params
    inp("w1", [FFH, DM], bf16)            # ffn_w1 lhsT, m-tiled rows
    inp("w2", [DM, FFH], bf16)            # ffn_w2 lhsT, m-tiled rows

    out = nc.dram_tensor("out", [DM, TS], f32, kind="ExternalOutput")

    scratch = dict(
        ar_in=nc.dram_tensor("ar_in", [NXD, L], f32),
        ar_out=nc.dram_tensor("ar_out", [NXD, L], f32),
        **{f"rs_in{q}": nc.dram_tensor(f"rs_in{q}", [GROUP * DM // 4, TS],
                                       bf16) for q in range(4)},
        **{f"rs_out{q}": nc.dram_tensor(f"rs_out{q}", [DM // 4, TS], bf16)
           for q in range(4)},
        bc_bf=nc.dram_tensor("bc_bf", [2 * NST, L], bf16),
        st1=nc.dram_tensor("st1", [2, L], f32),
        st2=nc.dram_tensor("st2", [2, TS], f32),
    )

    with tile.TileContext(nc, pool_alloc_mode="queue") as tc:
        _body(tc, nc, D, out, scratch, replica_groups)
    nc.compile()
    return nc


def _body(tc, nc, D, out, S, groups):
    from contextlib import ExitStack
    with ExitStack() as ctx:
        wp = ctx.enter_context(tc.tile_pool(name="wp", bufs=1))
        work = ctx.enter_context(tc.tile_pool(name="work", bufs=1))

        # ---- persistent weights in SBUF ----
        def wload(pool, name, shape, dt, rearr=None):
            t = pool.tile(shape, dt, name=name + "_sb")
            src = D[name].rearrange(rearr, p=P) if rearr else D[name][:]
            nc.sync.dma_start(out=t, in_=src)
            return t

        smalls = wload(wp, "smalls", [P, SM_COLS], f32)

        def sm(name, idx=None):
            off, ncols = _SM_OFF[name]
            if idx is None:
                assert ncols == 1
                return smalls[:, off:off + 1]
            return smalls[:, off + idx:off + idx + 1]

        def smblk(name):
            off, ncols = _SM_OFF[name]
            return smalls[:, off:off + ncols]

        ones_bf = wp.tile([P, 1], bf16)
        nc.vector.memset(ones_bf, 1.0)
        ones_row = wp.tile([1, P], bf16)
        nc.vector.memset(ones_row, 1.0)
        eps1 = wp.tile([1, 1], f32)
        nc.vector.memset(eps1, EPS)
        onef = wp.tile([P, 1], f32)
        nc.vector.memset(onef, 1.0)
        ident = wp.tile([P, P], bf16)
        nc.scalar.copy(out=ident, in_=smblk("ident"))

        # Pre-place activation-table loads: set 6 (natural_log_exp_and_
        # others) covers BOTH Exp and Ln, so the whole kernel needs only
        # 6 -> silu(18, auto-inserted) -> 6 instead of 8 greedy loads.
        def load_table(set_id):
            return nc.scalar.add_instruction(mybir.InstLoadActFuncSet(
                name=nc.get_next_instruction_name(),
                act_func_set_id=set_id, ins=[], outs=[]))

        _l6a = load_table(6)

        def ln_stats(load, n, ntiles, st_dram, psp, _exp_insts=None,
                     fuse_mu_r=False, dep0=None):
            if _exp_insts is None:
                _exp_insts = []
            """Column stats over partitions x ntiles; returns [P,2,n] tile
            (mean row 0, rstd row 1) broadcast via one DRAM round-trip."""
            nhh = (n + 511) // 512
            sum_x = [psp.tile([1, min(512, n)], f32, tag=f"sum_x{h}",
                              name=f"sum_x{h}") for h in range(nhh)]
            sum_sq = [psp.tile([1, min(512, n)], f32, tag=f"sum_sq{h}",
                               name=f"sum_sq{h}") for h in range(nhh)]
            for d in range(ntiles):
                xbf = load(d)  # [P, n] bf16 AP
                sq = work.tile([P, n], bf16, tag="stq", bufs=2, name="sq")
                nc.vector.tensor_mul(sq, xbf, xbf)
                for h in range(nhh):
                    sl = slice(h * 512, min((h + 1) * 512, n))
                    nc.tensor.matmul(sum_x[h], ones_bf, xbf[:, sl],
                                     start=(d == 0), stop=(d == ntiles - 1))
                    nc.tensor.matmul(sum_sq[h], ones_bf, sq[:, sl],
                                     start=(d == 0), stop=(d == ntiles - 1))
            # st[0,:]=mean, st[1,:]=rstd packed in one tile; per-half
            # roundtrip+broadcast so consumers can start on half 0 early.
            st = work.tile([1, 2, n], f32, tag="stat", bufs=1, name="st")
            msq = work.tile([1, n], f32, tag="stat2", bufs=1, name="msq")
            var = work.tile([1, n], f32, tag="stat3", bufs=1, name="var")
            lnv = work.tile([1, n], f32, tag="stat4", bufs=1, name="lnv")
            mr_bc = work.tile([P, 2, n], f32, tag="mr_bc", name="mr_bc")
            for h in range(nhh):
                sl = slice(h * 512, min((h + 1) * 512, n))
                nc.vector.tensor_scalar_mul(st[:, 0, sl], sum_x[h], 1.0 / DM)
                nc.vector.tensor_mul(msq[:, sl], st[:, 0, sl], st[:, 0, sl])
                nc.vector.scalar_tensor_tensor(var[:, sl], sum_sq[h],
                                               1.0 / DM, msq[:, sl],
                                               OP.mult, OP.subtract)
            # batch Ln ops then Exp ops (greedy table selection thrashes on
            # Ln<->Exp alternation: exp_and_others lacks Ln, natural_log
            # lacks Exp)
            for h in range(nhh):
                sl = slice(h * 512, min((h + 1) * 512, n))
                _li = nc.scalar.activation(lnv[:, sl], var[:, sl], AF.Ln,
                                           bias=eps1[:, 0:1])
                if dep0 is not None:
                    from concourse.bass import _add_dep_helper as _adh
                    _adh(_li.ins, dep0.ins, sync=False,
                         reason="ln after table load")
                    dep0 = None
            for h in range(nhh):
                sl = slice(h * 512, min((h + 1) * 512, n))
                w = sl.stop - sl.start
                _exp_insts.append(nc.scalar.activation(
                    st[:, 1, sl], lnv[:, sl], AF.Exp, scale=-0.5))
                if fuse_mu_r:  # row0 := mu*rstd for the folded-LN fixup
                    nc.vector.tensor_mul(st[:, 0, sl], st[:, 0, sl],
                                         st[:, 1, sl])
                # on-chip broadcast (Pool is idle here; frees the DMA queues)
                nc.gpsimd.partition_broadcast(mr_bc[:, :, sl],
                                              st[:, :, sl])
            return mr_bc

        xT_r = D["xT"].rearrange("(c p) t -> p c t", p=P)

        with ExitStack() as big_ctx:
            bigz = big_ctx.enter_context(tc.tile_pool(name="bigz", bufs=1))
            big = big_ctx.enter_context(tc.tile_pool(name="big", bufs=1))
            sz_bf = bigz.tile([P, DST, L], bf16)         # silu(z)
            xc_bf = big.tile([P, DST, L], bf16)          # silu(conv(xi))
            dt_bf = big.tile([P, DST, L], bf16)
            u_bf = big.tile([P, DST, L], bf16)           # dt * xc (bf16)
            # n=0 dA tiles, computed inside phase E right after each
            # channel tile's softplus: the first scan starts ~8us earlier
            dA0s = [big.tile([P, L], bf16, name=f"dA0_{i}")
                    for i in range(DST)]

            with ExitStack() as mid_ctx:
                mid = mid_ctx.enter_context(
                    tc.tile_pool(name="mid", bufs=1))
                xi_pad = mid.tile([P, DST, DCONV - 1 + L], bf16)

                # === Phase A/B: in_proj on RAW x, LN1 folded into W ===
                # W@LN1(x) = r_t*(Wg@x) - (mu_t r_t)*(W@g) + W@b;
                # host supplies Wg (w_in), vneg=-(W@g), w0 (xi part folded
                # into conv bias, z part is the silu ACT bias).
                with ExitStack() as ab_ctx:
                    xtp = ab_ctx.enter_context(
                        tc.tile_pool(name="xtp", bufs=1))
                    pss = ab_ctx.enter_context(
                        tc.tile_pool(name="pss", bufs=1, space="PSUM"))
                    psAB = ab_ctx.enter_context(
                        tc.tile_pool(name="psAB", bufs=4, space="PSUM"))
                    xbf_t = xtp.tile([P, DMT, L], bf16)
                    # 4 chunks on alternating queues so LN1 stats matmuls
                    # start on chunk 0 early
                    _x_r = D["xT_bf"].rearrange("(c p) t -> c p t", p=P)
                    for _c4 in range(4):
                        _cs = slice(_c4 * 2, (_c4 + 1) * 2)
                        _xq = nc.sync if _c4 % 2 == 0 else nc.scalar
                        _xq.dma_start(
                            out=xbf_t[:, _cs, :],
                            in_=_x_r[_cs].rearrange("c p t -> p c t"))
                    _z_silus = []
                    w_in_r = D["w_in"].rearrange(
                        "(m p) (c q) -> m p c q", p=P, q=P)

                    _ln1_exps = []
                    mr1 = ln_stats(lambda d: xbf_t[:, d, :], L, DMT,
                                   S["st1"], pss, _ln1_exps, fuse_mu_r=True,
                                   dep0=_l6a)
                    # mr1 rows: [0]=mu*r broadcast, [1]=r broadcast

                    w_x = wload(wp, "w_x", [P, DST, NXD], bf16,
                                "(c p) m -> p c m")
                    w_dt = wload(wp, "w_dt", [DTR, DS], bf16)

                    def in_proj_m(m):
                        w_in = xtp.tile([P, DMT, P], bf16, tag="w_in",
                                        bufs=3, name="w_in")
                        nc.sync.dma_start(out=w_in, in_=w_in_r[m])
                        for h in range(NH):
                            sl = slice(h * 512, (h + 1) * 512)
                            pt = psAB.tile([P, 512], f32, tag="mm")
                            for k in range(DMT):
                                nc.tensor.matmul(
                                    pt, w_in[:, k, :],
                                    xbf_t[:, k, sl],
                                    start=(k == 0), stop=(k == DMT - 1))
                            t1 = work.tile([P, 512], f32, tag="w4k",
                                           bufs=2, name="t1")
                            nc.vector.tensor_mul(t1, pt, mr1[:, 1, sl])
                            if m < DST:
                                nc.vector.scalar_tensor_tensor(
                                    xi_pad[:, m, DCONV - 1 + h * 512:
                                           DCONV - 1 + (h + 1) * 512],
                                    mr1[:, 0, sl], sm("vneg", m), t1,
                                    OP.mult, OP.add)
                            else:
                                zt = work.tile([P, 512], f32, tag="w2k",
                                               bufs=2, name="zt")
                                nc.vector.scalar_tensor_tensor(
                                    zt, mr1[:, 0, sl], sm("vneg", m), t1,
                                    OP.mult, OP.add)
                                _zs = nc.scalar.activation(
                                    sz_bf[:, m - DST, sl], zt, AF.Silu,
                                    bias=sm("w0z", m - DST))
                                _z_silus.append(_zs)
                                if _ln1_exps and _ln1_exps[0] is not None:
                                    from concourse.bass import \
                                        _add_dep_helper
                                    for _e in _ln1_exps:
                                        _add_dep_helper(
                                            _zs.ins, _e.ins, sync=False,
                                            reason="silu after ln1 exps")
                                    _ln1_exps.clear()
                                    _ln1_exps.append(None)

                    # xi half first: conv/x_proj/AllReduce fire before the
                    # z half keeps PE busy during the AR round-trip
                    for m in range(DST):
                        in_proj_m(m)

                    # ===== Phase C: depthwise conv (PE diag matmuls) =====
                    diags = []
                    for i in range(DST):
                        for k in range(DCONV):
                            dw = mid.tile([P, P], bf16, tag="dw", bufs=16,
                                          name="dw")
                            nc.vector.tensor_scalar(dw, ident,
                                                    sm("cw", i * DCONV + k),
                                                    None, OP.mult)
                            diags.append(dw)
                    # diag(D): folds the D*xc skip term into the scan's
                    # PSUM accumulation (kills the phase-G stt)
                    diag_D = []
                    for i in range(DST):
                        dD = bigz.tile([P, P], bf16, tag="dD", bufs=4,
                                       name="dD")
                        nc.vector.tensor_scalar(dD, ident, sm("Dvec", i),
                                                None, OP.mult)
                        diag_D.append(dD)
                    _conv_silus = []
                    for i in range(DST):
                        nc.vector.memset(xi_pad[:, i, 0:DCONV - 1], 0.0)
                        for h in range(NH):
                            s0 = h * 512
                            pt = psAB.tile([P, 512], f32, tag="mm")
                            for k in range(DCONV):
                                nc.tensor.matmul(
                                    pt, diags[i * DCONV + k],
                                    xi_pad[:, i, k + s0:k + s0 + 512],
                                    start=(k == 0), stop=(k == DCONV - 1))
                            _conv_silus.append(nc.scalar.activation(
                                xc_bf[:, i, s0:s0 + 512], pt,
                                AF.Silu, bias=sm("cb", i)))

                    # ===== Phase D: x_proj partial + AllReduce =====
                    for h in range(NH):
                        sl = slice(h * 512, (h + 1) * 512)
                        pt = psAB.tile([NXD, 512], f32, tag="mm")
                        for k in range(DST):
                            nc.tensor.matmul(pt, w_x[:, k, :],
                                             xc_bf[:, k, sl],
                                             start=(k == 0),
                                             stop=(k == DST - 1))
                        xd = work.tile([NXD, 512], f32, tag="w2k", bufs=2,
                                       name="xd")
                        nc.scalar.copy(out=xd, in_=pt)
                        nc.gpsimd.dma_start(out=S["ar_in"][:, sl], in_=xd)
                    if len(groups[0]) == 1:  # single-core sim variant
                        nc.sync.dma_start(out=S["ar_out"][:],
                                          in_=S["ar_in"][:])
                    else:
                        nc.gpsimd.collective_compute(
                            "AllReduce", OP.add, replica_groups=groups,
                            ins=[S["ar_in"][:]], outs=[S["ar_out"][:]])

                    # z half of in_proj overlaps the AllReduce round-trip
                    for m in range(DST, 2 * DST):
                        in_proj_m(m)

                # back to exp/ln table after the last silu
                from concourse.bass import _add_dep_helper
                _l6 = load_table(6)
                for _s in _conv_silus + _z_silus:
                    _add_dep_helper(_l6.ins, _s.ins, sync=False,
                                    reason="table load after silus")
                psDE = mid_ctx.enter_context(
                    tc.tile_pool(name="psDE", bufs=4, space="PSUM"))
                arst = mid_ctx.enter_context(
                    tc.tile_pool(name="arst", bufs=1))
                dt_low = arst.tile([DTR, L], bf16)
                dt_low32 = arst.tile([DTR, L], f32)
                bc32 = arst.tile([2 * NST, L], f32)
                bc16 = arst.tile([2 * NST, L], bf16)
                nc.sync.dma_start(out=dt_low32, in_=S["ar_out"][0:DTR, :])
                nc.vector.tensor_copy(dt_low, dt_low32)
                nc.sync.dma_start(out=bc32, in_=S["ar_out"][DTR:NXD, :])
                nc.vector.tensor_copy(bc16, bc32)
                nc.gpsimd.dma_start(out=S["bc_bf"][:], in_=bc16)

                # ======= Phase E: dt_proj + softplus; u = dt*xc =======
                # full-row [P,1024] activations: half the Act-queue latency
                # before the first scan can start
                for i in range(DST):
                    pt2 = psDE.tile([P, L], f32, tag="mm2", bufs=2)
                    for h in range(NH):
                        sl = slice(h * 512, (h + 1) * 512)
                        nc.tensor.matmul(pt2[:, sl],
                                         w_dt[:, i * P:(i + 1) * P],
                                         dt_low[:, sl],
                                         start=True, stop=True)
                    dte = arst.tile([P, L], f32, tag="dte", bufs=2,
                                    name="dte")
                    _de = nc.scalar.activation(dte, pt2, AF.Exp,
                                               bias=sm("dtb", i))
                    if _l6 is not None:
                        _add_dep_helper(_de.ins, _l6.ins, sync=False,
                                        reason="exp after table load")
                        _l6 = None
                    nc.scalar.activation(dt_bf[:, i, :], dte,
                                         AF.Ln, bias=onef[:, 0:1])
                    nc.scalar.activation(dA0s[i], dt_bf[:, i, :], AF.Exp,
                                         scale=sm("Amat", i * NST))
                    nc.vector.tensor_mul(u_bf[:, i, :], dt_bf[:, i, :],
                                         xc_bf[:, i, :])

            # ======= Phase F: selective scan over the 16 states =======
            # w1 first half prefetches during the scan (DMA is idle here)
            w_out = wload(wp, "w_out", [P, DST, DM], bf16,
                          "(c p) m -> p c m")
            xrs = wp.tile([P, DMT, TS], f32, name="xrs")
            nc.sync.dma_start(out=xrs,
                              in_=D["xTs"].rearrange("(c p) t -> p c t",
                                                     p=P))
            W1A = 16  # prefetched w1 m-tiles; the rest streams in FFN1
            ffw1a = ctx.enter_context(
                tc.tile_pool(name="ffw1a", bufs=1, side="right"))
            w1a = ffw1a.tile([P, W1A, DMT, P], bf16)
            # chunk DMAs, not one monolith: the critical dt/BC loads after
            # the AllReduce interleave between chunks
            _w1_r8 = D["w1"].rearrange("(m p) (c q) -> m p c q", p=P, q=P)
            for _c8 in range(W1A // 2):
                _ms = slice(_c8 * 2, (_c8 + 1) * 2)
                nc.scalar.dma_start(
                    out=w1a[:, _ms, :, :],
                    in_=_w1_r8[_ms].rearrange("m p c q -> p m c q"))

            with ExitStack() as scan_ctx:
                psS = scan_ctx.enter_context(
                    tc.tile_pool(name="psS", bufs=1, space="PSUM"))
                accs = [psS.tile([P, 512], f32, name=f"acc{i}h{h}")
                        for i in range(DST) for h in range(NH)]
                # seed each accumulator with D*xc via a diag matmul
                for i in range(DST):
                    for h in range(NH):
                        sl = slice(h * 512, (h + 1) * 512)
                        nc.tensor.matmul(accs[i * NH + h], diag_D[i],
                                         xc_bf[:, i, sl],
                                         start=True, stop=False)
                with tc.tile_pool(name="stream", bufs=2) as stream:
                    for n in range(NST):
                        # separate B/C tiles: B (read first, by DVE) frees
                        # its ring slot independently of C (read last, by
                        # Pool), killing a periodic dBx stall
                        Bn = stream.tile([P, L], bf16, tag="Bn", bufs=2)
                        src = S["bc_bf"][n:n + 1, :]
                        nc.sync.dma_start(out=Bn, in_=bass.AP(
                            tensor=src.tensor, offset=src.offset,
                            ap=[[0, P], [1, L]]))
                        Cn = stream.tile([P, L], bf16, tag="Cn", bufs=2)
                        src2 = S["bc_bf"][NST + n:NST + n + 1, :]
                        nc.scalar.dma_start(out=Cn, in_=bass.AP(
                            tensor=src2.tensor, offset=src2.offset,
                            ap=[[0, P], [1, L]]))
                        for i in range(DST):
                            if n == 0:
                                dA = dA0s[i]
                            else:
                                dA = stream.tile([P, L], bf16, tag="dA",
                                                 bufs=3)
                                nc.scalar.activation(dA, dt_bf[:, i, :],
                                                     AF.Exp,
                                                     scale=sm("Amat",
                                                              i * NST + n))
                            dBx = stream.tile([P, L], bf16, tag="dBx",
                                              bufs=3)
                            nc.vector.tensor_mul(dBx, u_bf[:, i, :], Bn)
                            hh = stream.tile([P, L], bf16, tag="h", bufs=3)
                            # scans are DVE-only on HW (TensorScalarPtr)
                            nc.vector.tensor_tensor_scan(hh, dA, dBx, 0.0,
                                                         OP.mult, OP.add)
                            prod = stream.tile([P, L], bf16, tag="prod",
                                               bufs=3)
                            # most C*h muls go to Pool to unload DVE
                            if (n * DST + i) % 6 == 5:
                                nc.vector.tensor_mul(prod, hh, Cn)
                            else:
                                nc.gpsimd.tensor_tensor(out=prod, in0=hh,
                                                        in1=Cn,
                                                        op=OP.mult)
                            for h in range(NH):
                                sl = slice(h * 512, (h + 1) * 512)
                                nc.tensor.matmul(
                                    accs[i * NH + h], ident, prod[:, sl],
                                    start=False, stop=(n == NST - 1))

                # tail pools open after the scan: stream/arst space is free
                tailp = ctx.enter_context(
                    tc.tile_pool(name="tailp", bufs=1, side="right"))
                y_bfs = [tailp.tile([P, L], bf16, name=f"y_bf{i}")
                         for i in range(DST)]
                o1 = tailp.tile([P, DMT, TS], f32)
                mrs = tailp.tile([P, DMT, TS], bf16, name="mrs")
                mr2b = tailp.tile([P, 2, TS], f32, name="mr2b")
                tail2 = ctx.enter_context(
                    tc.tile_pool(name="tail2", bufs=1, side="right"))
                xn2_bfs = [tail2.tile([P, TS], bf16, name=f"xn2_{d}")
                           for d in range(DMT)]
                h1_bf = tail2.tile([P, FFT, TS], bf16)

                # ======= Phase G: y = acc * silu(z) (Pool can't see PSUM)
                for i in range(DST):
                    for h in range(NH):
                        sl = slice(h * 512, (h + 1) * 512)
                        nc.vector.tensor_mul(y_bfs[i][:, sl],
                                             accs[i * NH + h],
                                             sz_bf[:, i, sl])

        # ======= Phase H: out_proj partial + quarter ReduceScatters ======
        # dm-quarters: each RS fires after 4 psum-drain groups, so the
        # LN2 stats pipeline starts while later quarters still compute.
        QD = DM // 4
        with ExitStack() as h_ctx:
            psH = h_ctx.enter_context(
                tc.tile_pool(name="psH", bufs=6, space="PSUM"))
            for q in range(4):
                rs_in, rs_out = S[f"rs_in{q}"], S[f"rs_out{q}"]
                rs_in_g = rs_in.rearrange("(g m) t -> g m t", g=GROUP)
                for mi in range(2):
                    m = q * 2 + mi
                    for h in range(NH):
                        sl = slice(h * 512, (h + 1) * 512)
                        pt = psH.tile([P, 512], f32, tag="mm")
                        for k in range(DST):
                            nc.tensor.matmul(pt,
                                             w_out[:, k, m * P:(m + 1) * P],
                                             y_bfs[k][:, sl],
                                             start=(k == 0),
                                             stop=(k == DST - 1))
                        ob = work.tile([P, 2, TS], bf16, tag="ob", bufs=6,
                                       name="ob")
                        # alternate copy engine + DMA queue to halve the
                        # PSUM-drain chain
                        if mi % 2 == 0:
                            nc.scalar.copy(
                                out=ob, in_=pt.rearrange("p (j t) -> p j t",
                                                         j=2))
                            dq = nc.scalar
                        else:
                            nc.vector.tensor_copy(
                                ob, pt.rearrange("p (j t) -> p j t", j=2))
                            dq = nc.sync
                        dq.dma_start(
                            out=rs_in_g[2 * h:2 * h + 2, mi * P:(mi + 1) * P,
                                        :].rearrange("j p t -> p j t"),
                            in_=ob)
                if len(groups[0]) == 1:  # single-core sim variant
                    nc.sync.dma_start(out=rs_out[:], in_=rs_in[0:QD, :])
                else:
                    nc.gpsimd.collective_compute("ReduceScatter", OP.add,
                                                 replica_groups=groups,
                                                 ins=[rs_in[:]],
                                                 outs=[rs_out[:]])


        # FFN pools open BEFORE LN2's psum pool: the PSUM stack otherwise
        # makes the first FFN matmul wait for LN2's pool close.
        j_ctx = ctx.enter_context(ExitStack())
        ffw = j_ctx.enter_context(
            tc.tile_pool(name="ffw", bufs=1, side="right"))
        psJ = j_ctx.enter_context(
            tc.tile_pool(name="psJ", bufs=4, space="PSUM"))

        # ======= Phase I: residual + LN2 on this core's token slice =======
        # Joint x/x^2 sum matmuls; stats broadcast across partitions via a
        # ones-row matmul into PSUM (no DRAM round-trip).
        with ExitStack() as ln2_ctx:
            pss2 = ln2_ctx.enter_context(
                tc.tile_pool(name="pss2", bufs=1, space="PSUM"))
            # gpsimd queue: don't serialize behind the RS copies on sync
            for q in range(4):
                nc.gpsimd.dma_start(
                    out=mrs[:, q * 2:(q + 1) * 2, :],
                    in_=S[f"rs_out{q}"].rearrange("(c p) t -> p c t", p=P))
            sums2 = pss2.tile([1, 2, TS], f32, name="sums2")
            for d in range(DMT):
                nc.vector.tensor_add(o1[:, d, :], xrs[:, d, :],
                                     mrs[:, d, :])
                xs = work.tile([P, 2, TS], bf16, tag="stq", bufs=2,
                               name="xs")
                nc.scalar.copy(out=xs[:, 0, :], in_=o1[:, d, :])
                nc.vector.tensor_mul(xs[:, 1, :], xs[:, 0, :], xs[:, 0, :])
                nc.tensor.matmul(sums2, ones_bf, xs,
                                 start=(d == 0), stop=(d == DMT - 1))
            st2t = work.tile([1, 2, TS], f32, tag="stat", bufs=1,
                             name="st2t")
            msq2 = work.tile([1, TS], f32, tag="stat2", bufs=1, name="msq2")
            var2 = work.tile([1, TS], f32, tag="stat3", bufs=1, name="var2")
            lnv2 = work.tile([1, TS], f32, tag="stat4", bufs=1, name="lnv2")
            nc.vector.tensor_scalar_mul(st2t[:, 0, :], sums2[:, 0, :],
                                        1.0 / DM)
            nc.vector.tensor_mul(msq2, st2t[:, 0, :], st2t[:, 0, :])
            nc.vector.scalar_tensor_tensor(var2, sums2[:, 1, :], 1.0 / DM,
                                           msq2, OP.mult, OP.subtract)
            nc.scalar.activation(lnv2, var2, AF.Ln, bias=eps1[:, 0:1])
            nc.scalar.activation(st2t[:, 1, :], lnv2, AF.Exp, scale=-0.5)
            nc.gpsimd.partition_broadcast(mr2b, st2t)
            for d in range(DMT):
                t1 = work.tile([P, TS], f32, tag="w2k", bufs=2, name="t2")
                nc.vector.tensor_sub(t1, o1[:, d, :], mr2b[:, 0, :])
                nc.vector.tensor_mul(t1, t1, mr2b[:, 1, :])
                nc.vector.tensor_scalar(xn2_bfs[d], t1,
                                        sm("ln2g", d), sm("ln2b", d),
                                        OP.mult, OP.add)

        # ======= Phase J: FFN (w1 half preloaded, rest streamed) =======
        w1_r = D["w1"].rearrange("(m p) (c q) -> m p c q", p=P, q=P)
        w2_r = D["w2"].rearrange("(m p) (c q) -> m p c q", p=P, q=P)
        out_r = out.rearrange("(c p) t -> c p t", p=P)
        if True:
            for m in range(FFT):
                if m < W1A:
                    w1s = w1a[:, m, :, :]
                else:
                    w1s = ffw.tile([P, DMT, P], bf16, tag="w1s", bufs=6,
                                   name="w1s")
                    nc.sync.dma_start(out=w1s, in_=w1_r[m])
                pt = psJ.tile([P, TS], f32, tag="mm")
                for k in range(DMT):
                    nc.tensor.matmul(pt, w1s[:, k, :], xn2_bfs[k],
                                     start=(k == 0), stop=(k == DMT - 1))
                nc.scalar.activation(h1_bf[:, m, :], pt, AF.Relu,
                                     bias=sm("b1", m))
            for m in range(DMT):
                # stream w2 per half-tile: finer grain keeps PE fed
                w2h = []
                for half in range(2):
                    w2s = ffw.tile([P, FFT // 2, P], bf16, tag="w2s",
                                   bufs=4, name="w2s")
                    nc.sync.dma_start(
                        out=w2s,
                        in_=w2_r[m][:, half * (FFT // 2):
                                    (half + 1) * (FFT // 2), :])
                    w2h.append(w2s)
                pt = psJ.tile([P, TS], f32, tag="mm")
                for k in range(FFT):
                    nc.tensor.matmul(pt, w2h[k // (FFT // 2)]
                                     [:, k % (FFT // 2), :], h1_bf[:, k, :],
                                     start=(k == 0), stop=(k == FFT - 1))
                o2m = ffw.tile([P, TS], f32, tag="o2m", bufs=2, name="o2m")
                nc.vector.scalar_tensor_tensor(o2m, pt, sm("b2", m),
                                               o1[:, m, :], OP.add, OP.add)
                nc.sync.dma_start(out=out_r[m], in_=o2m)


# ---------------- host side ----------------

_RUNNER = None


def _prep_core_inputs(inputs, c):
    b, s = divmod(c, GROUP)
    cs = slice(s * DS, (s + 1) * DS)
    ts = slice(s * TS, (s + 1) * TS)
    f = lambda a: np.ascontiguousarray(a, dtype=np.float32)
    h = lambda a: np.ascontiguousarray(a).astype(BF)
    xT = f(inputs["x"][b].T)
    in_w = np.asarray(inputs["in_proj_w"], dtype=np.float32)
    g1 = np.asarray(inputs["ln1_g"], np.float32)
    b1v = np.asarray(inputs["ln1_b"], np.float32)
    W_sel = np.concatenate([in_w[cs], in_w[DI:][cs]], axis=0)  # [2DS, DM]
    w_in_lhsT = (W_sel * g1[None, :]).T      # lhsT of W diag(g)
    v = W_sel @ g1
    w0 = W_sel @ b1v

    def mtile(lhsT):
        """[K, M] lhsT -> row-tiled [M, K] layout: row (m*P+p) = lhsT[
        : , m*P: ].reshape -> [c,q] flat; DMA slice per m is contiguous."""
        K, M = lhsT.shape
        return np.ascontiguousarray(
            lhsT.reshape(K // P, P, M // P, P).transpose(2, 1, 0, 3)
            .reshape(M, K))
    smalls = np.zeros((P, SM_COLS), np.float32)

    def put(name, arr):
        off, ncols = _SM_OFF[name]
        smalls[:, off:off + ncols] = arr.reshape(-1, P).T if arr.ndim == 1 \
            else arr

    cw_sum = np.asarray(inputs["conv_w"][cs, 0, :], np.float32).sum(axis=1)
    put("cb", np.asarray(inputs["conv_b"][cs], np.float32) + w0[:DS] * cw_sum)
    put("vneg", -v)
    put("w0z", w0[DS:])
    put("dtb", inputs["dt_proj_b"][cs])
    put("Dvec", inputs["D"][cs])
    put("ln1g", inputs["ln1_g"]); put("ln1b", inputs["ln1_b"])
    put("ln2g", inputs["ln2_g"]); put("ln2b", inputs["ln2_b"])
    put("b2", inputs["ffn_b2"])
    put("b1", inputs["ffn_b1"])
    put("ident", np.eye(P, dtype=np.float32))
    # cw: [p, c*DCONV + k] = conv_w[c*P + p, 0, k]
    cwm = np.asarray(inputs["conv_w"][cs, 0, :]).reshape(DST, P, DCONV)
    put("cw", cwm.transpose(1, 0, 2).reshape(P, DST * DCONV))
    Am = (-np.exp(np.asarray(inputs["A_log"][cs]))).reshape(DST, P, NST)
    put("Amat", Am.transpose(1, 0, 2).reshape(P, DST * NST))
    return {
        "xT": xT,
        "xT_bf": np.ascontiguousarray(xT).astype(BF),
        "xTs": f(xT[:, ts]),
        "w_in": h(mtile(w_in_lhsT)),
        "w_x": h(inputs["x_proj_w"][:, cs].T),
        "w_dt": h(inputs["dt_proj_w"][cs, :].T),
        "w_out": h(inputs["out_proj_w"][:, cs].T),
        "smalls": smalls,
        "w1": h(mtile(np.asarray(inputs["ffn_w1"], np.float32).T)),
        "w2": h(mtile(np.asarray(inputs["ffn_w2"], np.float32).T)),
    }


def _build_runner():
    import jax
    from jax.sharding import Mesh, PartitionSpec
    from jax.experimental.shard_map import shard_map
    from concourse import bass2jax as b2j

    nc = build_nc()
    b2j.install_neuronx_cc_hook()
    partition_name = (nc.partition_id_tensor.name
                      if nc.partition_id_tensor else None)

    in_names, out_names, out_avals, zero_outs = [], [], [], []
    for alloc in nc.m.functions[0].allocations:
        if not isinstance(alloc, mybir.MemoryLocationSet):
            continue
        name = alloc.memorylocations[0].name
        if alloc.kind == "ExternalInput":
            if name != partition_name:
                in_names.append(name)
        elif alloc.kind == "ExternalOutput":
            out_names.append(name)
            shape = tuple(alloc.tensor_shape)
            dtype = mybir.dt.np(alloc.dtype)
            out_avals.append(jax.core.ShapedArray(shape, dtype))
            zero_outs.append(np.zeros(shape, dtype))
    n_params, n_outs = len(in_names), len(out_avals)
    all_in_names = list(in_names) + list(out_names)
    if partition_name is not None:
        all_in_names.append(partition_name)
    donate = tuple(range(n_params, n_params + n_outs))

    def _mamba_block_body(*args):
        operands = list(args)
        if partition_name is not None:
            operands.append(b2j.partition_id_tensor())
        return tuple(b2j._bass_exec_p.bind(
            *operands, out_avals=tuple(out_avals),
            in_names=tuple(all_in_names), out_names=tuple(out_names),
            lowering_input_output_aliases=(),
            sim_require_finite=False, sim_require_nnan=False, nc=nc))

    devices = jax.devices()[:NCORES]
    mesh = Mesh(np.asarray(devices), ("core",))
    sharded = jax.jit(
        shard_map(_mamba_block_body, mesh=mesh,
                  in_specs=(PartitionSpec("core"),) * (n_params + n_outs),
                  out_specs=(PartitionSpec("core"),) * n_outs,
                  check_rep=False),
        donate_argnums=donate, keep_unused=True)

    def run(in_maps):
        concat_in = [
            np.concatenate([np.asarray(in_maps[c][nm])
                            for c in range(NCORES)], axis=0)
            for nm in in_names]
        concat_zeros = [np.zeros((NCORES * z.shape[0], *z.shape[1:]), z.dtype)
                        for z in zero_outs]
        out_arrs = sharded(*concat_in, *concat_zeros)
        out_arrs = [np.asarray(a) for a in out_arrs]
        return [{nm: out_arrs[i].reshape(NCORES, *out_avals[i].shape)[c]
                 for i, nm in enumerate(out_names)}
                for c in range(NCORES)]

    return run


def get_runner():
    global _RUNNER
    if _RUNNER is None:
        _RUNNER = _build_runner()
    return _RUNNER


def kernel(**inputs):
    run = get_runner()
    in_maps = [_prep_core_inputs(inputs, c) for c in range(NCORES)]
    outs = run(in_maps)
    result = np.empty((B, L, DM), np.float32)
    for c in range(NCORES):
        b, s = divmod(c, GROUP)
        result[b, s * TS:(s + 1) * TS, :] = outs[c]["out"].T
    return result
